# revision 30
# baseline (speedup 1.0000x reference)
"""Multi-head attention forward (B=4, L=2048, E=1024, H=16) on 8 NeuronCores.

Sharding: core c handles batch b = c // 2 and head-group g = c % 2 (8 heads,
512 embed dims). Each core computes its QKV projections, attention, and a
partial out-projection over its 512 contraction dims; the host sums the two
partials per batch and adds the bias.

Host pre-processing: x (q/k/v) is shipped transposed [E, L] in bf16 and the
weights pre-transposed (wqkvT [E, 3FG] bf16, woutT [FG, E] bf16), so the
kernel does no on-chip transposes at all.

Schedule: attention is ACT(exp)-bound, so PE work from the other phases is
interleaved into it — q/k projections for head-pairs 1..3 are issued between
the first attention round's steps, and out-projection matmuls for l-group
``lg`` are drip-fed into round ``lg+1``'s attention groups.

Self-contained: only needs numpy + the concourse stack at /opt/trn_rl_repo.
"""

import os
import sys

import numpy as np

sys.path.insert(0, "/opt/trn_rl_repo")

import ml_dtypes  # noqa: E402

import concourse.bass as bass  # noqa: E402
import concourse.tile as tile  # noqa: E402
from concourse import bacc, mybir  # noqa: E402
from concourse import bass_utils  # noqa: E402

F32 = mybir.dt.float32
BF16 = mybir.dt.bfloat16
EXP = mybir.ActivationFunctionType.Exp

P = 128          # partitions
L = 2048         # sequence length
E = 1024         # embed dim
FG = 512         # per-core feature slice (8 heads x 64)
D = 64           # head dim
LT = L // P      # 16 l-tiles
LG = L // 512    # 4 l-groups of 512
EC = E // P      # 8 e-chunks (contraction tiles for projections)
FT = FG // P     # 4 f-tiles (head pairs)
SC = L // P      # 16 s-chunks
GRP = 3          # (sc, h) pairs per exp group


def _build():
    nc = bacc.Bacc("TRN2", target_bir_lowering=False, debug=False, num_devices=8)

    debug = bool(os.environ.get("MHA_DEBUG"))
    xqT_d = nc.dram_tensor("xqT", [E, L], BF16, kind="ExternalInput")
    xkT_d = nc.dram_tensor("xkT", [E, L], BF16, kind="ExternalInput")
    xvT_d = nc.dram_tensor("xvT", [E, L], BF16, kind="ExternalInput")
    wqkvT_d = nc.dram_tensor("wqkvT", [E, 3 * FG], BF16, kind="ExternalInput")
    woutT_d = nc.dram_tensor("woutT", [FG, E], BF16, kind="ExternalInput")
    out_d = nc.dram_tensor("out", [L, E], BF16, kind="ExternalOutput")
    if debug:
        dbg_sum = nc.dram_tensor("dbg_sum", [2, 512], F32,
                                 kind="ExternalOutput")
        dbg_avN = nc.dram_tensor("dbg_avN", [P, 512], BF16,
                                 kind="ExternalOutput")

    with tile.TileContext(nc) as tc:
        with (
            tc.tile_pool(name="const", bufs=1) as constp,
            tc.tile_pool(name="qkv", bufs=1) as qkvp,
        ):
            warm32 = constp.tile([P, 16], F32, tag="warm32", name="warm32")
            nc.vector.memset(warm32[:], 0.0)
            warm16 = constp.tile([P, 16], BF16, tag="warm16", name="warm16")
            nc.vector.tensor_copy(warm16[:], warm32[:])  # DVE cast warm-up
            warmA = constp.tile([P, 16], F32, tag="warmA", name="warmA")
            nc.scalar.copy(warmA[:], warm32[:])          # ACT warm-up
            warmG = constp.tile([P, 16], F32, tag="warmG", name="warmG")
            nc.gpsimd.memset(warmG[:], 0.0)              # POOL warm-up

            # persistent tensors
            wT = [qkvp.tile([P, 3 * FG], BF16, tag=f"wT{ec}", name=f"wT{ec}")
                  for ec in range(EC)]
            woT = [qkvp.tile([P, E], BF16, tag=f"wo{ec}", name=f"wo{ec}")
                   for ec in range(4)]
            qT = [qkvp.tile([P, L], BF16, tag=f"qT{i}", name=f"qT{i}")
                  for i in range(FT)]
            kT = [qkvp.tile([P, L], BF16, tag=f"kT{i}", name=f"kT{i}")
                  for i in range(FT)]
            avN = [qkvp.tile([P, L], BF16, tag=f"avN{i}", name=f"avN{i}")
                   for i in range(FT)]
            # AV stationary: per s-chunk, 4 pairs x 256 cols:
            #   [v_h0(64) | ones(1) | 0(63)]  -> av rows 0:64, sums row 64
            #   [0(32) | ones(1) | 0(31) | v_h1(64)] -> av rows 64:128, sums row 32
            vst = qkvp.tile([P, SC * 1024], BF16, tag="vst", name="vst")


            pat = constp.tile([P, 1024], BF16, tag="pat", name="pat")
            nc.gpsimd.memset(pat[:], 0.0)
            for pp in range(FT):
                nc.gpsimd.memset(pat[:, pp * 256 + 64: pp * 256 + 65], 1.0)
                nc.gpsimd.memset(pat[:, pp * 256 + 160: pp * 256 + 161], 1.0)
            for sc in range(SC):
                nc.vector.tensor_copy(vst[:, sc * 1024:(sc + 1) * 1024],
                                      pat[:])

            with (
                tc.tile_pool(name="attnT", bufs=3) as attp,
                tc.tile_pool(name="bc", bufs=2) as bcp,
            ):
                def load_x(xdram, key):
                    xt = [xTp.tile([P, L], BF16, tag=f"xT{ec}", name=f"xT{ec}")
                          for ec in range(EC)]
                    for ec in range(EC):
                        nc.sync.dma_start(
                            xt[ec][:], xdram.ap()[ec * P:(ec + 1) * P, :])
                    return xt

                def vproj(xt, aux):
                    for lt in range(LT):
                        ps = aux.tile([P, FG], F32, tag="aux", name="vps")
                        for ec in range(EC):
                            nc.tensor.matmul(
                                ps[:],
                                xt[ec][:, lt * P:(lt + 1) * P],
                                wT[ec][:, 2 * FG:3 * FG],
                                start=(ec == 0),
                                stop=(ec == EC - 1),
                            )
                        ps4 = ps[:].rearrange("p (a b) -> p a b", b=P)
                        vd = vst[:, lt * 1024:(lt + 1) * 1024].rearrange(
                            "p (a b) -> p a b", b=256)
                        nc.vector.tensor_copy(vd[:, :, 0:64], ps4[:, :, 0:64])
                        nc.vector.tensor_copy(vd[:, :, 192:256],
                                              ps4[:, :, 64:128])

                def qkproj(xt, ft, outT, wcol0, aux):
                    # one head-pair's projection: 4 groups of 8 matmuls
                    for lg in range(LG):
                        ps = aux.tile([P, 512], F32, tag="aux", name="qkps")
                        for ec in range(EC):
                            nc.tensor.matmul(
                                ps[:],
                                wT[ec][:, wcol0 + ft * P: wcol0 + (ft + 1) * P],
                                xt[ec][:, lg * 512:(lg + 1) * 512],
                                start=(ec == 0),
                                stop=(ec == EC - 1),
                            )
                        nc.vector.tensor_copy(
                            outT[ft][:, lg * 512:(lg + 1) * 512], ps[:])

                # ---------------- attention -------------------------------
                tails = []

                def attn_round(lg, ps_sc, ps_av):
                    for p in range(FT):
                        avA = ps_av.tile([P, 512], F32, tag="avA", name="avA")
                        avB = ps_av.tile([P, 512], F32, tag="avB", name="avB")
                        av_bank = (avA, avB)

                        def av_mms(t0, n, aT, p=p, av_bank=av_bank):
                            for j in range(n):
                                sc, h = divmod(t0 + j, 2)
                                nc.tensor.matmul(
                                    av_bank[h][:],
                                    vst[:, sc * 1024 + p * 256 + 128 * h:
                                        sc * 1024 + p * 256 + 128 * h + 128],
                                    aT[:, j * 512:(j + 1) * 512],
                                    start=(sc == 0),
                                    stop=(sc == SC - 1),
                                )

                        pending = []
                        for t0 in range(0, 2 * SC, GRP):
                            n = min(GRP, 2 * SC - t0)
                            sc_ps = ps_sc.tile([P, 512 * GRP], F32, tag="sc",
                                               name="sc")
                            for j in range(n):
                                sc, h = divmod(t0 + j, 2)
                                nc.tensor.matmul(
                                    sc_ps[:, j * 512:(j + 1) * 512],
                                    kT[p][64 * h:64 * h + 64,
                                          sc * P:(sc + 1) * P],
                                    qT[p][64 * h:64 * h + 64,
                                          lg * 512:(lg + 1) * 512],
                                    start=True, stop=True,
                                )
                            aT = attp.tile([P, 512 * GRP], BF16, tag="aT",
                                           name="aT")
                            nc.scalar.activation(aT[:, 0:512 * n],
                                                 sc_ps[:, 0:512 * n], EXP,
                                                 scale=0.125)
                            if t0 == 0 and tails:
                                tails.pop()()  # prev (lg,p) tail after this
                                # group's first exp is queued
                            # av runs 2 groups behind exp so round-boundary
                            # scores are never queued behind leftover avs
                            if len(pending) >= 2:
                                av_mms(*pending.pop(0))
                            pending.append((t0, n, aT))

                        def tail(avA=avA, avB=avB, p=p, lg=lg,
                                 pending=tuple(pending), av_mms=av_mms,
                                 split=False):
                            for args in pending:
                                av_mms(*args)
                            # drain av banks to SBUF fast (frees PSUM), then
                            # normalize off the critical path
                            avS0 = bcp.tile([P, 512], F32, tag="avS0",
                                            name="avS0")
                            nc.vector.tensor_copy(avS0[:], avA[:])
                            avS1 = bcp.tile([P, 512], F32, tag="avS1",
                                            name="avS1")
                            nc.vector.tensor_copy(avS1[:], avB[:])
                            def norm(avS0=avS0, avS1=avS1, p=p, lg=lg):
                                r0 = bcp.tile([1, 512], F32, tag="r0", name="r0",
                                               bufs=1)
                                nc.vector.tensor_copy(r0[:], avS0[64:65, :])
                                r1 = bcp.tile([1, 512], F32, tag="r1",
                                              name="r1", bufs=1)
                                nc.vector.tensor_copy(r1[:], avS1[32:33, :])
                                rr0 = bcp.tile([1, 512], F32, tag="rr0",
                                               name="rr0", bufs=1)
                                nc.vector.reciprocal_approx_fast(rr0[:],
                                                                 r0[:])
                                rr1 = bcp.tile([1, 512], F32, tag="rr1",
                                               name="rr1", bufs=1)
                                nc.vector.reciprocal_approx_fast(rr1[:],
                                                                 r1[:])
                                bc0 = bcp.tile([P, 512], F32, tag="bc0",
                                               name="bc0", bufs=1)
                                nc.gpsimd.partition_broadcast(bc0[:], rr0[:])
                                bc1 = bcp.tile([P, 512], F32, tag="bc1",
                                               name="bc1", bufs=1)
                                nc.gpsimd.partition_broadcast(bc1[:], rr1[:])
                                nc.vector.tensor_mul(
                                    avN[p][0:64, lg * 512:(lg + 1) * 512],
                                    avS0[0:64, :], bc0[0:64, :])
                                nc.gpsimd.tensor_mul(
                                    avN[p][64:128, lg * 512:(lg + 1) * 512],
                                    avS1[64:128, :], bc1[64:128, :])
                                if debug and p == 0 and lg == 0:
                                    nc.sync.dma_start(dbg_sum.ap()[0:1, :],
                                                      avS0[64:65, :])
                                    nc.sync.dma_start(dbg_sum.ap()[1:2, :],
                                                      avS1[32:33, :])
                                    nc.sync.dma_start(
                                        dbg_avN.ap(), avN[0][:, 0:512])
                            if split:
                                return norm
                            norm()
                        tails.append(tail)

                with (
                    tc.tile_pool(name="xT", bufs=2) as xTp,
                    tc.tile_pool(name="ps1", bufs=4, space="PSUM") as aux1,
                ):
                    def load_x(xdram):
                        xt = [xTp.tile([P, L], BF16, tag=f"xT{ec}",
                                       name=f"xT{ec}") for ec in range(EC)]
                        for ec in range(EC):
                            nc.sync.dma_start(
                                xt[ec][:], xdram.ap()[ec * P:(ec + 1) * P, :])
                        return xt

                    # interleave wT and xv loads: the ec-th v-proj matmul
                    # needs exactly wT[ec] + xv[ec]
                    xtv = [xTp.tile([P, L], BF16, tag=f"xT{ec}",
                                    name=f"xT{ec}") for ec in range(EC)]
                    for ec in range(EC):
                        nc.sync.dma_start(
                            wT[ec][:], wqkvT_d.ap()[ec * P:(ec + 1) * P, :])
                        nc.sync.dma_start(
                            xtv[ec][:], xvT_d.ap()[ec * P:(ec + 1) * P, :])
                    vproj(xtv, aux1)
                    xtq = load_x(xqT_d)
                    xtk = load_x(xkT_d)
                    for ec in range(4):
                        nc.sync.dma_start(
                            woT[ec][:], woutT_d.ap()[ec * P:(ec + 1) * P, :])
                    for ft in range(FT):
                        qkproj(xtq, ft, qT, 0, aux1)
                        qkproj(xtk, ft, kT, FG, aux1)

                with (
                    tc.tile_pool(name="ps_sc", bufs=2, space="PSUM") as ps_sc,
                    tc.tile_pool(name="ps_av", bufs=1, space="PSUM") as ps_av,
                ):
                    for lg in range(LG):
                        attn_round(lg, ps_sc, ps_av)
                    last_norm = tails.pop()(split=True)
                    assert not tails

                with (
                    tc.tile_pool(name="ost", bufs=3) as ost,
                    tc.tile_pool(name="ps3", bufs=4, space="PSUM") as aux,
                ):

                    # ------------ output projection -----------------------
                    # SBUF-only normalize of the final (3,3) tail runs
                    # here, overlapping the lt 0..11 matmuls (only lt 12..15
                    # read the avN block it produces)
                    last_norm()
                    for lt in range(LT):
                        osb = ost.tile([P, E], BF16, tag="osb", name="osb")
                        psA = aux.tile([P, 512], F32, tag="aux", name="psA")
                        psB = aux.tile([P, 512], F32, tag="aux", name="psB")
                        for ec in range(4):
                            # both halves back-to-back: stationary reused
                            nc.tensor.matmul(
                                psA[:], avN[ec][:, lt * P:(lt + 1) * P],
                                woT[ec][:, 0:512],
                                start=(ec == 0), stop=(ec == 3))
                            nc.tensor.matmul(
                                psB[:], avN[ec][:, lt * P:(lt + 1) * P],
                                woT[ec][:, 512:1024],
                                start=(ec == 0), stop=(ec == 3))
                        nc.scalar.copy(osb[:, 0:512], psA[:])
                        nc.vector.tensor_copy(osb[:, 512:1024], psB[:])
                        nc.sync.dma_start(
                            out_d.ap()[lt * P:(lt + 1) * P, :], osb[:])

    nc.compile()
    return nc


_NC = None


def _get_nc():
    global _NC
    if _NC is None:
        _NC = _build()
    return _NC


def _shard_inputs(query, key, value, in_proj_weight, out_proj_weight):
    bf16 = ml_dtypes.bfloat16
    # per-batch transposed bf16 activations (shared by the two head-group
    # cores of each batch)
    xT = []
    for b in range(4):
        xT.append((
            np.ascontiguousarray(query[b].T.astype(bf16)),
            np.ascontiguousarray(key[b].T.astype(bf16)),
            np.ascontiguousarray(value[b].T.astype(bf16)),
        ))
    in_maps = []
    for c in range(8):
        b, g = divmod(c, 2)
        sl = slice(FG * g, FG * g + FG)
        wq = in_proj_weight[0 * E:1 * E][sl]
        wk = in_proj_weight[1 * E:2 * E][sl]
        wv = in_proj_weight[2 * E:3 * E][sl]
        wqkvT = np.ascontiguousarray(
            np.concatenate([wq, wk, wv], axis=0).T.astype(bf16))
        woutT = np.ascontiguousarray(
            out_proj_weight[:, sl].T.astype(bf16))
        in_maps.append({
            "xqT": xT[b][0],
            "xkT": xT[b][1],
            "xvT": xT[b][2],
            "wqkvT": wqkvT,
            "woutT": woutT,
        })
    return in_maps


def run_sharded(in_maps, **kwargs):
    nc = _get_nc()
    return bass_utils.run_bass_kernel_spmd(
        nc, in_maps, core_ids=list(range(8)), **kwargs)


def kernel(query, key, value, in_proj_weight, out_proj_weight, out_proj_bias):
    query = np.asarray(query, dtype=np.float32)
    key = np.asarray(key, dtype=np.float32)
    value = np.asarray(value, dtype=np.float32)
    in_proj_weight = np.asarray(in_proj_weight, dtype=np.float32)
    out_proj_weight = np.asarray(out_proj_weight, dtype=np.float32)
    out_proj_bias = np.asarray(out_proj_bias, dtype=np.float32)

    in_maps = _shard_inputs(query, key, value, in_proj_weight, out_proj_weight)
    res = run_sharded(in_maps)
    out = np.empty((4, L, E), dtype=np.float32)
    for b in range(4):
        out[b] = (res.results[2 * b]["out"].astype(np.float32)
                  + res.results[2 * b + 1]["out"].astype(np.float32))
    out += out_proj_bias
    return out


# revision 32
# speedup vs baseline: 1.0065x; 1.0065x over previous
"""Multi-head attention forward (B=4, L=2048, E=1024, H=16) on 8 NeuronCores.

Sharding: core c handles batch b = c // 2 and head-group g = c % 2 (8 heads,
512 embed dims). Each core computes its QKV projections, attention, and a
partial out-projection over its 512 contraction dims; the host sums the two
partials per batch and adds the bias.

Host pre-processing: x (q/k/v) is shipped transposed [E, L] in bf16 and the
weights pre-transposed (wqkvT [E, 3FG] bf16, woutT [FG, E] bf16), so the
kernel does no on-chip transposes at all.

Schedule: attention is ACT(exp)-bound, so PE work from the other phases is
interleaved into it — q/k projections for head-pairs 1..3 are issued between
the first attention round's steps, and out-projection matmuls for l-group
``lg`` are drip-fed into round ``lg+1``'s attention groups.

Self-contained: only needs numpy + the concourse stack at /opt/trn_rl_repo.
"""

import os
import sys

import numpy as np

sys.path.insert(0, "/opt/trn_rl_repo")

import ml_dtypes  # noqa: E402

import concourse.bass as bass  # noqa: E402
import concourse.tile as tile  # noqa: E402
from concourse import bacc, mybir  # noqa: E402
from concourse import bass_utils  # noqa: E402

F32 = mybir.dt.float32
BF16 = mybir.dt.bfloat16
EXP = mybir.ActivationFunctionType.Exp

P = 128          # partitions
L = 2048         # sequence length
E = 1024         # embed dim
FG = 512         # per-core feature slice (8 heads x 64)
D = 64           # head dim
LT = L // P      # 16 l-tiles
LG = L // 512    # 4 l-groups of 512
EC = E // P      # 8 e-chunks (contraction tiles for projections)
FT = FG // P     # 4 f-tiles (head pairs)
SC = L // P      # 16 s-chunks
GRP = 3          # (sc, h) pairs per exp group


def _build():
    nc = bacc.Bacc("TRN2", target_bir_lowering=False, debug=False, num_devices=8)

    debug = bool(os.environ.get("MHA_DEBUG"))
    xqT_d = nc.dram_tensor("xqT", [E, L], BF16, kind="ExternalInput")
    xkT_d = nc.dram_tensor("xkT", [E, L], BF16, kind="ExternalInput")
    xvT_d = nc.dram_tensor("xvT", [E, L], BF16, kind="ExternalInput")
    wqkvT_d = nc.dram_tensor("wqkvT", [E, 3 * FG], BF16, kind="ExternalInput")
    woutT_d = nc.dram_tensor("woutT", [FG, E], BF16, kind="ExternalInput")
    out_d = nc.dram_tensor("out", [L, E], BF16, kind="ExternalOutput")
    if debug:
        dbg_sum = nc.dram_tensor("dbg_sum", [2, 512], F32,
                                 kind="ExternalOutput")
        dbg_avN = nc.dram_tensor("dbg_avN", [P, 512], BF16,
                                 kind="ExternalOutput")

    with tile.TileContext(nc) as tc:
        with (
            tc.tile_pool(name="const", bufs=1) as constp,
            tc.tile_pool(name="qkv", bufs=1) as qkvp,
        ):
            warm32 = constp.tile([P, 16], F32, tag="warm32", name="warm32")
            nc.vector.memset(warm32[:], 0.0)
            warm16 = constp.tile([P, 16], BF16, tag="warm16", name="warm16")
            nc.vector.tensor_copy(warm16[:], warm32[:])  # DVE cast warm-up
            warmA = constp.tile([P, 16], F32, tag="warmA", name="warmA")
            nc.scalar.copy(warmA[:], warm32[:])          # ACT warm-up
            warmG = constp.tile([P, 16], F32, tag="warmG", name="warmG")
            nc.gpsimd.memset(warmG[:], 0.0)              # POOL warm-up

            # persistent tensors
            wT = [qkvp.tile([P, 3 * FG], BF16, tag=f"wT{ec}", name=f"wT{ec}")
                  for ec in range(EC)]
            woT = [qkvp.tile([P, E], BF16, tag=f"wo{ec}", name=f"wo{ec}")
                   for ec in range(4)]
            qT = [qkvp.tile([P, L], BF16, tag=f"qT{i}", name=f"qT{i}")
                  for i in range(FT)]
            kT = [qkvp.tile([P, L], BF16, tag=f"kT{i}", name=f"kT{i}")
                  for i in range(FT)]
            avN = [qkvp.tile([P, L], BF16, tag=f"avN{i}", name=f"avN{i}")
                   for i in range(FT)]
            # AV stationary: per s-chunk, 4 pairs x 256 cols:
            #   [v_h0(64) | ones(1) | 0(63)]  -> av rows 0:64, sums row 64
            #   [0(32) | ones(1) | 0(31) | v_h1(64)] -> av rows 64:128, sums row 32
            vst = qkvp.tile([P, SC * 1024], BF16, tag="vst", name="vst")


            pat = constp.tile([P, 1024], BF16, tag="pat", name="pat")
            nc.gpsimd.memset(pat[:], 0.0)
            for pp in range(FT):
                nc.gpsimd.memset(pat[:, pp * 256 + 64: pp * 256 + 65], 1.0)
                nc.gpsimd.memset(pat[:, pp * 256 + 160: pp * 256 + 161], 1.0)
            for sc in range(SC):
                nc.vector.tensor_copy(vst[:, sc * 1024:(sc + 1) * 1024],
                                      pat[:])

            with (
                tc.tile_pool(name="attnT", bufs=3) as attp,
                tc.tile_pool(name="bc", bufs=2) as bcp,
            ):
                def load_x(xdram, key):
                    xt = [xTp.tile([P, L], BF16, tag=f"xT{ec}", name=f"xT{ec}")
                          for ec in range(EC)]
                    for ec in range(EC):
                        nc.sync.dma_start(
                            xt[ec][:], xdram.ap()[ec * P:(ec + 1) * P, :])
                    return xt

                def vproj(xt, aux):
                    for lt in range(LT):
                        ps = aux.tile([P, FG], F32, tag="aux", name="vps")
                        for ec in range(EC):
                            nc.tensor.matmul(
                                ps[:],
                                xt[ec][:, lt * P:(lt + 1) * P],
                                wT[ec][:, 2 * FG:3 * FG],
                                start=(ec == 0),
                                stop=(ec == EC - 1),
                            )
                        ps4 = ps[:].rearrange("p (a b) -> p a b", b=P)
                        vd = vst[:, lt * 1024:(lt + 1) * 1024].rearrange(
                            "p (a b) -> p a b", b=256)
                        nc.vector.tensor_copy(vd[:, :, 0:64], ps4[:, :, 0:64])
                        nc.vector.tensor_copy(vd[:, :, 192:256],
                                              ps4[:, :, 64:128])

                def qkproj(xt, ft, outT, wcol0, aux):
                    # one head-pair's projection: 4 groups of 8 matmuls
                    for lg in range(LG):
                        ps = aux.tile([P, 512], F32, tag="aux", name="qkps")
                        for ec in range(EC):
                            nc.tensor.matmul(
                                ps[:],
                                wT[ec][:, wcol0 + ft * P: wcol0 + (ft + 1) * P],
                                xt[ec][:, lg * 512:(lg + 1) * 512],
                                start=(ec == 0),
                                stop=(ec == EC - 1),
                            )
                        nc.vector.tensor_copy(
                            outT[ft][:, lg * 512:(lg + 1) * 512], ps[:])

                # ---------------- attention -------------------------------
                tails = []

                def attn_round(lg, ps_sc, ps_av):
                    for p in range(FT):
                        avA = ps_av.tile([P, 512], F32, tag="avA", name="avA")
                        avB = ps_av.tile([P, 512], F32, tag="avB", name="avB")
                        av_bank = (avA, avB)

                        def av_mms(t0, n, aT, p=p, av_bank=av_bank):
                            for j in range(n):
                                sc, h = divmod(t0 + j, 2)
                                nc.tensor.matmul(
                                    av_bank[h][:],
                                    vst[:, sc * 1024 + p * 256 + 128 * h:
                                        sc * 1024 + p * 256 + 128 * h + 128],
                                    aT[:, j * 512:(j + 1) * 512],
                                    start=(sc == 0),
                                    stop=(sc == SC - 1),
                                )

                        pending = []
                        for t0 in range(0, 2 * SC, GRP):
                            n = min(GRP, 2 * SC - t0)
                            sc_ps = ps_sc.tile([P, 512 * GRP], F32, tag="sc",
                                               name="sc")
                            for j in range(n):
                                sc, h = divmod(t0 + j, 2)
                                nc.tensor.matmul(
                                    sc_ps[:, j * 512:(j + 1) * 512],
                                    kT[p][64 * h:64 * h + 64,
                                          sc * P:(sc + 1) * P],
                                    qT[p][64 * h:64 * h + 64,
                                          lg * 512:(lg + 1) * 512],
                                    start=True, stop=True,
                                )
                            aT = attp.tile([P, 512 * GRP], BF16, tag="aT",
                                           name="aT")
                            nc.scalar.activation(aT[:, 0:512 * n],
                                                 sc_ps[:, 0:512 * n], EXP,
                                                 scale=0.125)
                            if t0 == 0 and tails:
                                tails.pop()()  # prev (lg,p) tail after this
                                # group's first exp is queued
                            # av runs 2 groups behind exp so round-boundary
                            # scores are never queued behind leftover avs
                            if len(pending) >= 2:
                                av_mms(*pending.pop(0))
                            pending.append((t0, n, aT))

                        def tail(avA=avA, avB=avB, p=p, lg=lg,
                                 pending=tuple(pending), av_mms=av_mms):
                            for args in pending:
                                av_mms(*args)
                            # drain av banks to SBUF fast (frees PSUM), then
                            # normalize off the critical path
                            avS0 = bcp.tile([P, 512], F32, tag="avS0",
                                            name="avS0")
                            nc.vector.tensor_copy(avS0[:], avA[:])
                            avS1 = bcp.tile([P, 512], F32, tag="avS1",
                                            name="avS1")
                            nc.vector.tensor_copy(avS1[:], avB[:])
                            r0 = bcp.tile([1, 512], F32, tag="r0", name="r0",
                                          bufs=1)
                            nc.vector.tensor_copy(r0[:], avS0[64:65, :])
                            r1 = bcp.tile([1, 512], F32, tag="r1", name="r1",
                                          bufs=1)
                            nc.vector.tensor_copy(r1[:], avS1[32:33, :])
                            rr0 = bcp.tile([1, 512], F32, tag="rr0",
                                           name="rr0", bufs=1)
                            nc.vector.reciprocal_approx_fast(rr0[:], r0[:])
                            rr1 = bcp.tile([1, 512], F32, tag="rr1",
                                           name="rr1", bufs=1)
                            nc.vector.reciprocal_approx_fast(rr1[:], r1[:])
                            bc0 = bcp.tile([P, 512], F32, tag="bc0",
                                           name="bc0", bufs=1)
                            nc.gpsimd.partition_broadcast(bc0[:], rr0[:])
                            bc1 = bcp.tile([P, 512], F32, tag="bc1",
                                           name="bc1", bufs=1)
                            nc.gpsimd.partition_broadcast(bc1[:], rr1[:])
                            nc.vector.tensor_mul(
                                avN[p][0:64, lg * 512:(lg + 1) * 512],
                                avS0[0:64, :], bc0[0:64, :])
                            nc.gpsimd.tensor_mul(
                                avN[p][64:128, lg * 512:(lg + 1) * 512],
                                avS1[64:128, :], bc1[64:128, :])
                            if debug and p == 0 and lg == 0:
                                nc.sync.dma_start(dbg_sum.ap()[0:1, :],
                                                  avS0[64:65, :])
                                nc.sync.dma_start(dbg_sum.ap()[1:2, :],
                                                  avS1[32:33, :])
                                nc.sync.dma_start(
                                    dbg_avN.ap(), avN[0][:, 0:512])
                        tails.append(tail)

                with (
                    tc.tile_pool(name="xT", bufs=2) as xTp,
                    tc.tile_pool(name="ps1", bufs=4, space="PSUM") as aux1,
                ):
                    def load_x(xdram):
                        xt = [xTp.tile([P, L], BF16, tag=f"xT{ec}",
                                       name=f"xT{ec}") for ec in range(EC)]
                        for ec in range(EC):
                            nc.sync.dma_start(
                                xt[ec][:], xdram.ap()[ec * P:(ec + 1) * P, :])
                        return xt

                    # interleave wT and xv loads: the ec-th v-proj matmul
                    # needs exactly wT[ec] + xv[ec]
                    xtv = [xTp.tile([P, L], BF16, tag=f"xT{ec}",
                                    name=f"xT{ec}") for ec in range(EC)]
                    for ec in range(EC):
                        nc.sync.dma_start(
                            wT[ec][:], wqkvT_d.ap()[ec * P:(ec + 1) * P, :])
                        nc.sync.dma_start(
                            xtv[ec][:], xvT_d.ap()[ec * P:(ec + 1) * P, :])
                    vproj(xtv, aux1)
                    xtq = load_x(xqT_d)
                    xtk = load_x(xkT_d)
                    for ec in range(4):
                        nc.sync.dma_start(
                            woT[ec][:], woutT_d.ap()[ec * P:(ec + 1) * P, :])
                    for ft in range(FT):
                        qkproj(xtq, ft, qT, 0, aux1)
                        qkproj(xtk, ft, kT, FG, aux1)

                with (
                    tc.tile_pool(name="ps_sc", bufs=2, space="PSUM") as ps_sc,
                    tc.tile_pool(name="ps_av", bufs=1, space="PSUM") as ps_av,
                    tc.tile_pool(name="ost", bufs=3) as ost,
                ):
                    for lg in range(LG):
                        attn_round(lg, ps_sc, ps_av)
                    while tails:
                        tails.pop()()

                    # ---- output projection: reuses the sc psum tiles so no
                    # pool transition (and its drain barrier) is needed ----
                    for lt in range(LT):
                        osb = ost.tile([P, E], BF16, tag="osb", name="osb")
                        pso = ps_sc.tile([P, 512 * GRP], F32, tag="sc",
                                         name="pso")
                        for ec in range(4):
                            # both halves back-to-back: stationary reused
                            nc.tensor.matmul(
                                pso[:, 0:512],
                                avN[ec][:, lt * P:(lt + 1) * P],
                                woT[ec][:, 0:512],
                                start=(ec == 0), stop=(ec == 3))
                            nc.tensor.matmul(
                                pso[:, 512:1024],
                                avN[ec][:, lt * P:(lt + 1) * P],
                                woT[ec][:, 512:1024],
                                start=(ec == 0), stop=(ec == 3))
                        nc.scalar.copy(osb[:, 0:512], pso[:, 0:512])
                        nc.vector.tensor_copy(osb[:, 512:1024],
                                              pso[:, 512:1024])
                        nc.sync.dma_start(
                            out_d.ap()[lt * P:(lt + 1) * P, :], osb[:])

    nc.compile()
    return nc


_NC = None


def _get_nc():
    global _NC
    if _NC is None:
        _NC = _build()
    return _NC


def _shard_inputs(query, key, value, in_proj_weight, out_proj_weight):
    bf16 = ml_dtypes.bfloat16
    # per-batch transposed bf16 activations (shared by the two head-group
    # cores of each batch)
    xT = []
    for b in range(4):
        xT.append((
            np.ascontiguousarray(query[b].T.astype(bf16)),
            np.ascontiguousarray(key[b].T.astype(bf16)),
            np.ascontiguousarray(value[b].T.astype(bf16)),
        ))
    in_maps = []
    for c in range(8):
        b, g = divmod(c, 2)
        sl = slice(FG * g, FG * g + FG)
        wq = in_proj_weight[0 * E:1 * E][sl]
        wk = in_proj_weight[1 * E:2 * E][sl]
        wv = in_proj_weight[2 * E:3 * E][sl]
        wqkvT = np.ascontiguousarray(
            np.concatenate([wq, wk, wv], axis=0).T.astype(bf16))
        woutT = np.ascontiguousarray(
            out_proj_weight[:, sl].T.astype(bf16))
        in_maps.append({
            "xqT": xT[b][0],
            "xkT": xT[b][1],
            "xvT": xT[b][2],
            "wqkvT": wqkvT,
            "woutT": woutT,
        })
    return in_maps


def run_sharded(in_maps, **kwargs):
    nc = _get_nc()
    return bass_utils.run_bass_kernel_spmd(
        nc, in_maps, core_ids=list(range(8)), **kwargs)


def kernel(query, key, value, in_proj_weight, out_proj_weight, out_proj_bias):
    query = np.asarray(query, dtype=np.float32)
    key = np.asarray(key, dtype=np.float32)
    value = np.asarray(value, dtype=np.float32)
    in_proj_weight = np.asarray(in_proj_weight, dtype=np.float32)
    out_proj_weight = np.asarray(out_proj_weight, dtype=np.float32)
    out_proj_bias = np.asarray(out_proj_bias, dtype=np.float32)

    in_maps = _shard_inputs(query, key, value, in_proj_weight, out_proj_weight)
    res = run_sharded(in_maps)
    out = np.empty((4, L, E), dtype=np.float32)
    for b in range(4):
        out[b] = (res.results[2 * b]["out"].astype(np.float32)
                  + res.results[2 * b + 1]["out"].astype(np.float32))
    out += out_proj_bias
    return out
